# revision 1
# baseline (speedup 1.0000x reference)
"""Weighted-KNN (retrieval_knn) Trainium2 kernel.

Math (per query c, over N anchors):
    sq[n]   = ||c - p_n||^2 / (w_n^2 + eps)
    top-8 smallest sq -> softmax(-sq_k / TEMP) -> weighted sum of features.

Per core (data-parallel over B across 8 cores), per 128-query tile:
  * y[q, n] = -sq[q,n]/TEMP on TensorE as a rank-5 inner product over
    centered coordinates (c' = c - 0.5, p' = p - 0.5), float32r
    single-pass matmuls (4x the fp32 rate; selection tolerates the
    reduced precision because candidates are re-scored exactly),
    4 row-tiled groups, 16 supertiles of 1024 anchors.
  * Packed top-8: ScalarE drains each PSUM supertile as bf16 into the
    HIGH halves of a persistent [128, 16384] fp32 buffer whose LOW
    halves hold the global anchor id (initialized once).  A single
    fp32 DVE top-8 `max` over all 16384 packed words yields the top-8
    values AND ids in one 16384-cycle pass -- any reduction tree costs
    the same total DVE cycles, so the one-shot scan is optimal.
  * The 8 winners (256B rows: bf16 features + fp32 [p'x3, g0]) are
    fetched with one gpsimd dma_gather and re-scored EXACTLY from p',
    so matmul/bf16 rounding only perturbs near-ties whose softmax
    weights are nearly equal; softmax + bf16 tree weighted feature sum.
  * coords are preloaded to SBUF (no per-tile input DMA -> the SP DMA
    queue holds only output stores and cannot serialize tiles).
"""

import sys

if "/opt/trn_rl_repo" not in sys.path:
    sys.path.insert(0, "/opt/trn_rl_repo")

import numpy as np

import concourse.bacc as bacc
import concourse.bass as bass
import concourse.mybir as mybir
from concourse.bass import ts
from concourse.bass_utils import run_bass_kernel_spmd
from concourse.tile import TileContext

B, N, D, F = 65536, 16384, 3, 64
K = 8
BANDWIDTH = 0.05
TEMP = 2.0 * BANDWIDTH * BANDWIDTH  # 0.005
INV_TEMP = 1.0 / TEMP  # 200.0
EPS = 1e-8
NCORES = 8
Q = B // NCORES  # 8192 queries per core
P = 128
NT = Q // P  # 64 query tiles per core
CH = 512  # matmul free-dim chunk (one PSUM bank)
NG = 4  # row-tiled matmul groups (PE bands)
NGN = N // NG  # 4096 anchors per group
NST = 16  # supertiles of 1024 anchors
STW = 1024
CR = 64  # fp32 elems per comb row (256B): 32 feat(bf16x2) + 4 pw + pad
LOOP = 1  # in-NEFF repetitions of the tile loop (benchmarking)
import os as _os

STAGE = int(_os.environ.get("KNN_STAGE", "99"))  # 1=ids, 3=gather, 99=full
USE_FR = int(_os.environ.get("KNN_FR", "0"))  # f32r matmuls (fast, noisy)
DMA_PACKS = int(_os.environ.get("KNN_DMAPACK", "0"))  # (dead: DMA cannot read PSUM)

FP = mybir.dt.float32
FR = mybir.dt.float32r
BF = mybir.dt.bfloat16
I32 = mybir.dt.int32
I16 = mybir.dt.int16


def _build_nc():
    nc = bacc.Bacc("TRN2", num_swdge_queues=2)
    coords = nc.declare_dram_parameter("coords", [Q, D], FP, isOutput=False)
    positions = nc.declare_dram_parameter("positions", [N, D], FP, isOutput=False)
    weights = nc.declare_dram_parameter("weights", [N], FP, isOutput=False)
    features = nc.declare_dram_parameter("features", [N, F], FP, isOutput=False)
    ident_in = nc.declare_dram_parameter("ident", [P, P], FP, isOutput=False)
    perm_in = nc.declare_dram_parameter("perm", [P, 8, P], FP, isOutput=False)
    pkinit_in = nc.declare_dram_parameter("pkinit", [P, N], I32, isOutput=False)
    out = nc.declare_dram_parameter("out", [Q, F], FP, isOutput=True)

    # gather table row n: words 0..31 = 64 bf16 features,
    # words 32..35 = [p'0, p'1, p'2, -inv/TEMP], rest pad (256B rows).
    comb_hbm = nc.dram_tensor("comb_stage", [N, CR], FP)
    hstage = nc.dram_tensor("hstage", [3, P, 5], FP)

    with TileContext(nc) as tc:
        with (
            tc.tile_pool(name="const", bufs=1) as cpool,
            nc.gpsimd.register("nidx") as nidx_reg,
        ):
            nc.gpsimd.reg_mov(nidx_reg, P * K)

            ident = cpool.tile([P, P], FP)
            nc.sync.dma_start(ident[:], ident_in[:])
            pconst = cpool.tile([P, 8, P], FP)
            nc.sync.dma_start(pconst[:], perm_in[:])

            # G4[32m + r, j] = g_r[m*4096 + j] (separate f32r-rounded
            # copy only when f32r matmuls are enabled)
            G4 = cpool.tile([P, NGN], FP)
            G4r = cpool.tile([P, NGN], FR) if USE_FR else G4

            # all coords for this core: csb[p, t, :] = coords[t*128+p] - 0.5
            csb = cpool.tile([P, NT, D], FP)
            nc.sync.dma_start(csb[:], coords[:].rearrange("(t p) d -> p t d", p=P))
            nc.vector.tensor_scalar_add(csb[:], csb[:], -0.5)

            # persistent packed buffers; low halves = global anchor id
            # (init DMAs are emitted after the G4 build, below)
            pk = [cpool.tile([P, N], FP, name=f"pk{i}") for i in range(2)]

            # ---------------- prep: build G and the gather table ----------
            with tc.tile_pool(name="prep", bufs=2) as pp:
                pos_sb = pp.tile([P, P, D], FP)
                nc.sync.dma_start(
                    pos_sb[:], positions[:].rearrange("(p j) d -> p j d", p=P)
                )
                nc.vector.tensor_scalar_add(pos_sb[:], pos_sb[:], -0.5)
                w_sb = pp.tile([P, P], FP)
                nc.sync.dma_start(w_sb[:], weights[:].rearrange("(p j) -> p j", p=P))

                inv = pp.tile([P, P], FP)
                nc.vector.tensor_mul(inv[:], w_sb[:], w_sb[:])
                nc.vector.tensor_scalar_add(inv[:], inv[:], EPS)
                nc.vector.reciprocal(inv[:], inv[:])

                g0 = pp.tile([P, P], FP)
                nc.vector.tensor_scalar_mul(g0[:], inv[:], -INV_TEMP)

                gd = [
                    pp.tile([P, P], FP, tag=f"g{d + 1}", name=f"g{d + 1}")
                    for d in range(D)
                ]
                for d in range(D):
                    nc.vector.tensor_mul(gd[d][:], inv[:], pos_sb[:, :, d])
                    nc.vector.tensor_scalar_mul(gd[d][:], gd[d][:], 2.0 * INV_TEMP)

                pp2 = pp.tile([P, P], FP)
                tmp = pp.tile([P, P], FP)
                nc.vector.tensor_mul(pp2[:], pos_sb[:, :, 0], pos_sb[:, :, 0])
                nc.vector.tensor_mul(tmp[:], pos_sb[:, :, 1], pos_sb[:, :, 1])
                nc.vector.tensor_add(pp2[:], pp2[:], tmp[:])
                nc.vector.tensor_mul(tmp[:], pos_sb[:, :, 2], pos_sb[:, :, 2])
                nc.vector.tensor_add(pp2[:], pp2[:], tmp[:])
                g4c = pp.tile([P, P], FP)
                nc.vector.tensor_mul(g4c[:], g0[:], pp2[:])

                # scatter [128, 128] component tiles into G4 group lanes
                for r, comp in enumerate([g0, gd[0], gd[1], gd[2], g4c]):
                    for m in range(NG):
                        src = comp[32 * m : 32 * (m + 1), :]
                        dst = bass.AP(
                            G4[:].tensor,
                            (32 * m + r) * NGN,
                            [[NGN, 1], [P, 32], [1, P]],
                        )
                        nc.sync.dma_start(dst, src)
                if USE_FR:
                    nc.vector.tensor_copy(G4r[:], G4[:])

                # pk id inits: after the matmul-critical prep, on the gpsimd
                # DMA queue, so the first tiles' matmuls start immediately
                for i in range(2):
                    nc.gpsimd.dma_start(pk[i][:].bitcast(I32), pkinit_in[:])

                # pw quad of anchor n = 128 p + j -> comb[n, 32..36]
                pwt = pp.tile([P, P, 4], FP)
                for f, comp in enumerate(
                    [pos_sb[:, :, 0], pos_sb[:, :, 1], pos_sb[:, :, 2], g0[:]]
                ):
                    nc.vector.tensor_copy(pwt[:, :, f], comp)
                pw_dst = bass.AP(
                    comb_hbm[:].tensor, 32, [[P * CR, P], [CR, P], [1, 4]]
                )
                nc.sync.dma_start(pw_dst, pwt[:])

                # features -> bf16 rows; chunk ck: n = 2048 ck + 128 j + p
                feat_src = features[:].rearrange(
                    "(ck j p) f -> ck p j f", ck=8, j=16
                )
                comb_bf = comb_hbm[:].bitcast(BF).tensor
                for ck in range(8):
                    fsb = pp.tile([P, 16, F], FP, tag="fsb", name=f"fsb{ck}")
                    nc.sync.dma_start(fsb[:], feat_src[ck])
                    fsb16 = pp.tile([P, 16, F], BF, tag="fsb16", name=f"fsb16_{ck}")
                    nc.vector.tensor_copy(fsb16[:], fsb[:])
                    dst = bass.AP(
                        comb_bf,
                        2048 * ck * 2 * CR,
                        [[2 * CR, P], [P * 2 * CR, 16], [1, F]],
                    )
                    nc.gpsimd.dma_start(dst, fsb16[:])

            # ---------------- main loop over query tiles ----------------
            with (
                tc.tile_pool(name="mm_ps", bufs=3, space="PSUM") as pspool,
                tc.tile_pool(name="ht_ps", bufs=1, space="PSUM") as htpool,
                tc.tile_pool(name="pi_ps", bufs=1, space="PSUM") as pipool,
                tc.tile_pool(name="hs", bufs=3) as hpool,
                tc.tile_pool(name="sm", bufs=6) as sm,
                tc.tile_pool(name="g8", bufs=4) as gpool,
            ):
                def tail(tlp, tp, pkbp, nctp):
                        # --- packed top-8 per half: 16 candidates ---
                        v8 = sm.tile([P, 2, K], FP, tag="v8")
                        nc.vector.max(v8[:, 0, :], pkbp[:, 0 : N // 2])
                        nc.vector.max(v8[:, 1, :], pkbp[:, N // 2 : N])
                        aid = sm.tile([P, 2 * K], I32, tag="aid")
                        nc.vector.tensor_scalar(
                            aid[:],
                            v8[:].bitcast(I32),
                            65535,
                            None,
                            op0=mybir.AluOpType.bitwise_and,
                        )
                        idxf = sm.tile([P, 2 * K], FP, tag="idxf")
                        nc.scalar.copy(idxf[:], aid[:])

                        if STAGE == 1:
                            dump = sm.tile([P, F], FP, tag="dump", name=f"dump{tlp}")
                            nc.vector.memset(dump[:], 0.0)
                            nc.vector.tensor_copy(dump[:, 0:2*K], v8[:])
                            nc.vector.tensor_copy(dump[:, 2*K : 4*K], idxf[:])
                            nc.sync.dma_start(out[ts(tp, P), :], dump[:])
                            return

                        # wrapped int16 idx layout for dma_gather
                        psI = pipool.tile([P, 8, 2 * K], FP, tag="pitmp", name=f"psI_{tlp}")
                        for u in range(8):
                            nc.tensor.matmul(
                                psI[:, u, :],
                                pconst[:, u, :],
                                idxf[:],
                                start=True,
                                stop=True,
                            )
                        idxw = sm.tile([P, 2 * K * 8], I16, tag="idxw")
                        idxw_uk = bass.AP(idxw[:].tensor, 0, [[2 * K * 8, P], [1, 8], [8, 2 * K]])
                        nc.scalar.copy(idxw_uk, psI[:])

                        # --- gather the 8 winners (256B rows) per query ---
                        cg = gpool.tile([P, 2 * K, CR], FP, tag="cg")
                        for gh in range(2):
                            nc.gpsimd.dma_gather(
                                cg[:, gh * K : (gh + 1) * K, :],
                                comb_hbm[:],
                                idxw[:, gh * K * 8 : (gh + 1) * K * 8],
                                P * K,
                                nidx_reg,
                                CR,
                                queue_num=gh,
                            )

                        if STAGE == 3:
                            dump = sm.tile([P, F], FP, tag="dump", name=f"dump{tlp}")
                            nc.vector.tensor_copy(dump[:], cg[:, 0, :])
                            nc.sync.dma_start(out[ts(tp, P), :], dump[:])
                            return

                        # --- exact rescore: y = sum_d (p'_d - c'_d)^2 * g0 ---
                        sqd = [
                            sm.tile([P, 2 * K], FP, tag=f"sqd{d}", name=f"sqd{d}")
                            for d in range(D)
                        ]
                        for d in range(D):
                            ind = bass.AP(
                                cg[:].tensor, 32 + d, [[2 * K * CR, P], [CR, 2 * K]]
                            )
                            nc.scalar.activation(
                                sqd[d][:],
                                ind,
                                mybir.ActivationFunctionType.Square,
                                bias=nctp[:, d : d + 1],
                                scale=1.0,
                            )
                        nc.vector.tensor_add(sqd[0][:], sqd[0][:], sqd[1][:])
                        nc.vector.tensor_add(sqd[0][:], sqd[0][:], sqd[2][:])
                        y8 = sm.tile([P, 2 * K], FP, tag="y8")
                        g0v = bass.AP(cg[:].tensor, 32 + 3, [[2 * K * CR, P], [CR, 2 * K]])
                        nc.vector.tensor_mul(y8[:], sqd[0][:], g0v)

                        # --- exact top-8 of the 16 + masked softmax ---
                        v8x = sm.tile([P, K], FP, tag="v8x")
                        nc.vector.max(v8x[:], y8[:])
                        nv1 = sm.tile([P, 1], FP, tag="nv1")
                        nc.vector.tensor_scalar_mul(nv1[:], v8x[:, 0:1], -1.0)
                        e16 = sm.tile([P, 2 * K], FP, tag="e16")
                        nc.scalar.activation(
                            e16[:],
                            y8[:],
                            mybir.ActivationFunctionType.Exp,
                            bias=nv1[:],
                            scale=1.0,
                        )
                        ew = sm.tile([P, 2 * K], FP, tag="ew")
                        nc.vector.scalar_tensor_tensor(
                            ew[:],
                            y8[:],
                            v8x[:, K - 1 : K],
                            e16[:],
                            op0=mybir.AluOpType.is_ge,
                            op1=mybir.AluOpType.mult,
                        )
                        ssum = sm.tile([P, 1], FP, tag="ssum")
                        nc.vector.reduce_sum(
                            out=ssum[:], in_=ew[:], axis=mybir.AxisListType.X
                        )
                        rs = sm.tile([P, 1], FP, tag="rs")
                        nc.vector.reciprocal(rs[:], ssum[:])

                        # --- weighted sum of candidate features (bf16 tree) ---
                        # duplicate weights into bf16 pairs so the mul's
                        # broadcast rides a stride-0 MIDDLE dim with a packed
                        # [1,2] last dim -> 2x DVE mode
                        ewp = sm.tile([P, 2 * K, 2], BF, tag="ewp")
                        nc.scalar.copy(ewp[:, :, 0], ew[:])
                        nc.scalar.copy(ewp[:, :, 1], ew[:])
                        fe = gpool.tile([P, 2 * K, F], BF, tag="fe")
                        feats4 = bass.AP(
                            cg[:].bitcast(BF).tensor,
                            0,
                            [[4 * K * CR, P], [2 * CR, 2 * K], [2, F // 2], [1, 2]],
                        )
                        fe4 = bass.AP(
                            fe[:].tensor,
                            0,
                            [[2 * K * F, P], [F, 2 * K], [2, F // 2], [1, 2]],
                        )
                        ewp4 = bass.AP(
                            ewp[:].tensor,
                            0,
                            [[4 * K, P], [2, 2 * K], [0, F // 2], [1, 2]],
                        )
                        nc.vector.tensor_mul(fe4, feats4, ewp4)
                        half = 2 * K
                        while half > 1:
                            half //= 2
                            nc.vector.tensor_add(
                                fe[:, 0:half, :],
                                fe[:, 0:half, :],
                                fe[:, half : 2 * half, :],
                            )
                        ot = gpool.tile([P, F], FP, tag="ot")
                        nc.scalar.activation(
                            ot[:],
                            fe[:, 0, :],
                            mybir.ActivationFunctionType.Copy,
                            bias=0.0,
                            scale=rs[:],
                        )

                        nc.sync.dma_start(out[ts(tp, P), :], ot[:])


                nct_hist = [None, None, None]
                for tl in range(NT * LOOP):
                    t = tl % NT
                    pkb = pk[tl % 2]

                    # --- per-tile H build, replicated to 4 group lanes ---
                    ct = csb[:, t, :]
                    nct = hpool.tile([P, D], FP, tag="nct")
                    nc.vector.tensor_scalar_mul(nct[:], ct, -1.0)
                    nct_hist[tl % 3] = nct
                    cc = hpool.tile([P, D], FP, tag="cc")
                    nc.vector.tensor_mul(cc[:], ct, ct)
                    hsrc = hpool.tile([P, 5], FP, tag="hsrc")
                    nc.vector.reduce_sum(
                        out=hsrc[:, 0:1], in_=cc[:], axis=mybir.AxisListType.X
                    )
                    nc.vector.tensor_copy(hsrc[:, 1:4], ct)
                    nc.vector.memset(hsrc[:, 4:5], 1.0)
                    # transpose via a DRAM staging bounce (DRAM-side APs may
                    # iterate in any order): frees ScalarE and the PE.
                    hsx = tl % 3
                    hs_w = bass.AP(
                        hstage[:].tensor, hsx * P * 5, [[5, P], [1, 5]]
                    )
                    nc.gpsimd.dma_start(hs_w, hsrc[:])
                    hT4 = hpool.tile([P, P], FR if USE_FR else FP, tag="hT4")
                    for m in range(NG):
                        hs_r = bass.AP(
                            hstage[:].tensor, hsx * P * 5, [[1, 5], [5, P]]
                        )
                        nc.gpsimd.dma_start(hT4[32 * m : 32 * m + 5, :], hs_r)

                    # --- supertiles: matmul + bf16 pack into pk.  Issued in
                    # band-rotating triples (the 3-deep PSUM ring allows 3
                    # supertiles in flight) so consecutive matmuls hit three
                    # different 32-row PE bands and can overlap streaming.
                    groups = [(0, 4, 8), (12, 1, 5), (9, 13, 2),
                              (6, 10, 14), (3, 7, 11), (15,)]
                    for grp in groups:
                        sts = {
                            s: pspool.tile(
                                [P, STW], FP, tag="st", name=f"st{tl}_{s}"
                            )
                            for s in grp
                        }
                        for h in range(2):
                            for s in grp:
                                m = s >> 2
                                cbase = (s & 3) * STW
                                nc.tensor.matmul(
                                    sts[s][:, h * CH : (h + 1) * CH],
                                    hT4[32 * m : 32 * m + 5, :],
                                    G4r[
                                        32 * m : 32 * m + 5,
                                        cbase + h * CH : cbase + (h + 1) * CH,
                                    ],
                                    start=True,
                                    stop=True,
                                    tile_position=(32 * m, 0),
                                )
                        for s in grp:
                            hi = bass.AP(
                                pkb[:].bitcast(BF).tensor,
                                2 * (STW * s) + 1,
                                [[2 * N, P], [2, STW]],
                            )
                            if DMA_PACKS and s in (3, 7, 11, 15)[: DMA_PACKS]:
                                # strided 2-byte DMA moves the fp32 high
                                # halves (bf16 by truncation) off ScalarE
                                shi = bass.AP(
                                    sts[s][:].bitcast(BF).tensor,
                                    1,
                                    [[2 * STW, P], [2, STW]],
                                )
                                nc.gpsimd.dma_start(hi, shi)
                            else:
                                nc.scalar.copy(hi, sts[s][:])

                    if tl > 0:
                        tail(tl - 1, (tl - 1) % NT, pk[(tl - 1) % 2],
                             nct_hist[(tl - 1) % 3])

                nc_last = NT * LOOP - 1
                tail(nc_last, nc_last % NT, pk[nc_last % 2],
                     nct_hist[nc_last % 3])

    nc.compile()
    return nc


_NC = None
LAST_RESULT = None


def _host_consts():
    ident = np.eye(P, dtype=np.float32)
    perm = np.zeros((P, 8, P), dtype=np.float32)
    for u in range(8):
        for p16 in range(16):
            perm[16 * u + p16, u, p16::16] = 1.0
    pkinit = np.tile(np.arange(N, dtype=np.int32), (P, 1))
    return ident, perm, pkinit


def make_in_maps(inputs):
    coords = np.ascontiguousarray(inputs["coords"], dtype=np.float32)
    positions = np.ascontiguousarray(inputs["positions"], dtype=np.float32)
    weights = np.ascontiguousarray(inputs["weights"], dtype=np.float32)
    features = np.ascontiguousarray(inputs["features"], dtype=np.float32)
    ident, perm, pkinit = _host_consts()
    return [
        {
            "coords": coords[i * Q : (i + 1) * Q],
            "positions": positions,
            "weights": weights,
            "features": features,
            "ident": ident,
            "perm": perm,
            "pkinit": pkinit,
        }
        for i in range(NCORES)
    ]


def kernel(coords, positions, weights, features):
    global _NC, LAST_RESULT
    import os

    if _NC is None:
        _NC = _build_nc()

    in_maps = make_in_maps(
        {
            "coords": coords,
            "positions": positions,
            "weights": weights,
            "features": features,
        }
    )
    trace = bool(int(os.environ.get("KNN_TRACE", "0")))
    res = run_bass_kernel_spmd(_NC, in_maps, core_ids=list(range(NCORES)), trace=trace)
    LAST_RESULT = res
    return np.concatenate([res.results[i]["out"] for i in range(NCORES)], axis=0)

